# revision 12
# baseline (speedup 1.0000x reference)
"""GaussianMixture log-likelihood kernel for 8 TRN2 NeuronCores (v2).

Math (per point x, cluster k):
  S_k = L_k L_k^T  (L = cov_inv_sqrt),  coef_k = pr_k * |det L_k|
  d_ik = -0.5 x^T S_k x + x^T S_k c_k - 0.5 c_k^T S_k c_k
  ll_i = log sum_k coef_k exp(d_ik) - threshold

Device strategy (data-parallel over N, 8192 points/core), all transposes
done host-side (X uploaded as X^T, duplicated across both 64-partition
halves so the PE can run two row-tiled K=64 matmuls concurrently):

  - main MM (row-tiled x2): stationary = G chunk [64, 128] (columns =
    2 clusters x 64 whitened coords), moving = X^T [64, 512 pts]
    -> Z^T psum [128, 512].  Half A (rows 0-63) and half B (rows 64-127)
    run concurrently in separate PSUM banks.
  - evac+nonlinearize, split across two engines per-tile:
      ACT route: G = L columns,  z2 = Square(Z^T)          (scalar engine)
      DVE route: G = S columns,  wz = W^T * X^T_dup        (vector engine)
    both give SBUF bf16 tiles whose 64-partition segment sums = x^T S x.
  - reduce over the 64 coords via accumulating matmuls: stationary
    S_red[m] [128, 32] holds -0.5 at (row r, col 2m + r//64); 16 chunk
    matmuls accumulate d^T [32 k, 512 pts] into a col-tiled PSUM bank
    (4 point-groups per bank at col positions 0/32/64/96).  The linear
    term x.Sc is one more accumulating matmul into the same bank.
  - epilogue: E = exp(d^T + bias_k) (ACT, per-partition bias AP),
    sum over k via ones-block matmul into a packed s bank, ll = Ln(C*s)
    with C = exp(-EXPB - threshold) folded into the activation scale.
"""

import sys

sys.path.insert(0, "/opt/trn_rl_repo")

import numpy as np

from concourse import bacc, bass, mybir
from concourse.tile import TileContext
from concourse.bass_utils import run_bass_kernel_spmd

N, D, K = 65536, 64, 32
NCORES = 8
NLOC = N // NCORES            # 8192 points per core
HALF = NLOC // 2              # 4096 points per row-tile half
PG = 512                      # points per group (psum bank width)
NPG = HALF // PG              # 8 point-groups per half
NCH = K // 2                  # 16 chunks (2 clusters each)
EXPB = 60.0                   # exp bias: exp(d + EXPB), d <= ~0 always

F32 = mybir.dt.float32
BF16 = mybir.dt.bfloat16

# wg layout (columns, bf16): [G_L2 2048 | G_S2 2048 | ScM2 32 | S_red 16*32 | ones4 4]
WG_GL = 0
WG_GS = 2048
WG_SC = 4096
WG_SRED = 4128
WG_ONES = 4128 + 16 * 32
WG_COLS = WG_ONES + 4


def _build_nc(threshold_f: float):
    nc = bacc.Bacc()

    xa_d = nc.declare_dram_parameter("xa", [128, 2 * HALF], BF16, isOutput=False)
    wg_d = nc.declare_dram_parameter("wg", [128, WG_COLS], BF16, isOutput=False)
    bias_d = nc.declare_dram_parameter("biasv", [128, 1], F32, isOutput=False)
    out_d = nc.declare_dram_parameter("out", [4, 4, PG], F32, isOutput=True)

    lnscale = float(np.exp(-EXPB) * np.exp(-threshold_f))

    with TileContext(nc) as tc:
        with (
            tc.tile_pool(name="const", bufs=1) as cpool,
            tc.tile_pool(name="z2", bufs=8) as z2pool,
            tc.tile_pool(name="ee", bufs=2) as epool,
            tc.tile_pool(name="fin", bufs=1) as finpool,
            tc.tile_pool(name="psz", bufs=3, space="PSUM") as pszpool,
            tc.tile_pool(name="psd", bufs=2, space="PSUM") as psdpool,
            tc.tile_pool(name="pss", bufs=1, space="PSUM") as psspool,
        ):
            wg = cpool.tile([128, WG_COLS], BF16)
            # G matrices first so the first main-MM can start early
            nc.sync.dma_start(out=wg[:, 0:WG_SC], in_=wg_d[:, 0:WG_SC])
            nc.sync.dma_start(out=wg[:, WG_SC:WG_COLS], in_=wg_d[:, WG_SC:WG_COLS])
            biasv = cpool.tile([128, 1], F32)
            nc.sync.dma_start(out=biasv[:, :], in_=bias_d[:, :])

            xa = cpool.tile([128, 2 * HALF], BF16)
            for s in range(8):
                nc.sync.dma_start(
                    out=xa[:, s * 1024 : (s + 1) * 1024],
                    in_=xa_d[:, s * 1024 : (s + 1) * 1024],
                )

            GL = wg[:, WG_GL : WG_GL + 2048]
            GS = wg[:, WG_GS : WG_GS + 2048]
            ScM = wg[:, WG_SC : WG_SC + 32]
            ones4 = wg[:, WG_ONES : WG_ONES + 4]

            def sred(m):
                return wg[:, WG_SRED + m * 32 : WG_SRED + (m + 1) * 32]

            sbank = psspool.tile([128, PG], F32)
            nc.vector.memset(sbank[:, :], 0.0)

            tile_idx = 0
            for gpair in range(2):
                dbankA = psdpool.tile([128, PG], F32)
                dbankB = psdpool.tile([128, PG], F32)
                nc.vector.memset(dbankA[:, :], 0.0)
                nc.vector.memset(dbankB[:, :], 0.0)
                for pj in range(4):
                    p = gpair * 4 + pj
                    xsA = xa[:, p * PG : (p + 1) * PG]
                    xsB = xa[:, HALF + p * PG : HALF + (p + 1) * PG]
                    dA = dbankA[32 * pj : 32 * pj + 32, :]
                    dB = dbankB[32 * pj : 32 * pj + 32, :]
                    # linear term x.Sc starts each accumulation chain
                    nc.tensor.matmul(dA, ScM[0:64, :], xsA[0:64, :],
                                     start=False, stop=False,
                                     tile_position=(0, 32 * pj),
                                     skip_group_check=True)
                    nc.tensor.matmul(dB, ScM[64:128, :], xsB[64:128, :],
                                     start=False, stop=False,
                                     tile_position=(64, 32 * pj),
                                     skip_group_check=True)
                    for m in range(16):
                        for half, xs, dsl in ((0, xsA, dA), (1, xsB, dB)):
                            route_act = (tile_idx % 2) == 0
                            tile_idx += 1
                            G = GL if route_act else GS
                            zps = pszpool.tile([128, PG], F32)
                            nc.tensor.matmul(
                                zps,
                                G[64 * half : 64 * half + 64,
                                  m * 128 : (m + 1) * 128],
                                xs[64 * half : 64 * half + 64, :],
                                start=True, stop=True,
                            )
                            z2 = z2pool.tile([128, PG], BF16)
                            if route_act:
                                nc.scalar.activation(
                                    out=z2[:, :], in_=zps[:, :],
                                    func=mybir.ActivationFunctionType.Square,
                                )
                            else:
                                nc.vector.tensor_tensor(
                                    out=z2[:, :], in0=zps[:, :], in1=xs[:, :],
                                    op=mybir.AluOpType.mult,
                                )
                            nc.tensor.matmul(dsl, sred(m), z2[:, :],
                                             start=False, stop=(m == 15),
                                             tile_position=(0, 32 * pj),
                                             skip_group_check=True)
                # epilogue for this gpair: exp, then sum over k
                for half, dbank in ((0, dbankA), (1, dbankB)):
                    E = epool.tile([128, PG], BF16)
                    nc.scalar.activation(
                        out=E[:, :], in_=dbank[:, :],
                        func=mybir.ActivationFunctionType.Exp,
                        bias=biasv[:, 0:1],
                    )
                    colpos = 32 * (2 * half + gpair)
                    nc.tensor.matmul(
                        sbank[colpos : colpos + 4, :], ones4, E[:, :],
                        start=False, stop=True,
                        tile_position=(0, colpos),
                        skip_group_check=True,
                    )

            lnb = finpool.tile([128, PG], F32)
            nc.scalar.activation(
                out=lnb[:, :], in_=sbank[:, :],
                func=mybir.ActivationFunctionType.Ln,
            )
            llf = finpool.tile([128, PG], F32)
            nc.vector.tensor_scalar(
                out=llf[:, :], in0=lnb[:, :],
                scalar1=-(EXPB + threshold_f), scalar2=None,
                op0=mybir.AluOpType.add,
            )
            for j in range(4):
                nc.sync.dma_start(
                    out=out_d[j, :, :], in_=llf[32 * j : 32 * j + 4, :],
                )

    nc.compile()
    return nc


def _host_prep(X, center, cov_inv_sqrt, weight, threshold):
    L = cov_inv_sqrt.astype(np.float64)
    w = np.abs(weight.astype(np.float64))
    pr = w / w.sum()
    sign, logdetL = np.linalg.slogdet(L)
    logcoef = np.log(pr) + logdetL                       # [K]
    S = np.einsum("kde,kfe->kdf", L, L)                  # [K, D, D]
    Sc = np.einsum("kde,ke->kd", S, center.astype(np.float64))  # [K, D]
    cSc = np.einsum("kd,kd->k", center.astype(np.float64), Sc)  # [K]

    import ml_dtypes
    BFD = ml_dtypes.bfloat16

    # G_L / G_S: [64, 2048], column k*64+c
    G_L = np.ascontiguousarray(
        L.transpose(1, 0, 2).reshape(D, K * D))         # G_L[d, k*64+c] = L[k,d,c]
    G_S = np.ascontiguousarray(
        S.transpose(1, 0, 2).reshape(D, K * D))
    wg = np.zeros((128, WG_COLS), np.float64)
    wg[0:64, WG_GL:WG_GL + 2048] = G_L
    wg[64:128, WG_GL:WG_GL + 2048] = G_L
    wg[0:64, WG_GS:WG_GS + 2048] = G_S
    wg[64:128, WG_GS:WG_GS + 2048] = G_S
    wg[0:64, WG_SC:WG_SC + 32] = Sc.T                   # ScM[c, k]
    wg[64:128, WG_SC:WG_SC + 32] = Sc.T
    for m in range(NCH):
        blk = np.zeros((128, 32))
        blk[0:64, 2 * m] = -0.5
        blk[64:128, 2 * m + 1] = -0.5
        wg[:, WG_SRED + m * 32: WG_SRED + (m + 1) * 32] = blk
    for j in range(4):
        wg[32 * j: 32 * j + 32, WG_ONES + j] = 1.0

    biasv = np.tile(EXPB + logcoef - 0.5 * cSc, 4).astype(np.float32).reshape(128, 1)

    XT = np.ascontiguousarray(X.astype(np.float64).T)   # [64, N]
    thr = float(np.asarray(threshold, dtype=np.float64))
    return XT, wg.astype(BFD), biasv, thr


_CACHE = {}


def kernel(X, center, cov_inv_sqrt, weight, threshold):
    import ml_dtypes
    BFD = ml_dtypes.bfloat16

    XT, wg, biasv, thr = _host_prep(X, center, cov_inv_sqrt, weight, threshold)

    key = ("nc", thr)
    if key not in _CACHE:
        _CACHE[key] = _build_nc(thr)
    nc = _CACHE[key]

    in_maps = []
    for i in range(NCORES):
        xt = XT[:, i * NLOC : (i + 1) * NLOC]
        xa = np.zeros((128, 2 * HALF), np.float64)
        xa[0:64, 0:HALF] = xt[:, 0:HALF]
        xa[64:128, 0:HALF] = xt[:, 0:HALF]
        xa[0:64, HALF:] = xt[:, HALF:]
        xa[64:128, HALF:] = xt[:, HALF:]
        in_maps.append({"xa": xa.astype(BFD), "wg": wg, "biasv": biasv})

    res = run_bass_kernel_spmd(nc, in_maps, core_ids=list(range(NCORES)))
    outs = res.results
    ll = np.concatenate(
        [np.asarray(outs[i]["out"], dtype=np.float32).reshape(NLOC)
         for i in range(NCORES)]
    )
    return ll


# revision 16
# speedup vs baseline: 1.8162x; 1.8162x over previous
"""GaussianMixture log-likelihood kernel for 8 TRN2 NeuronCores (v2).

Math (per point x, cluster k):
  S_k = L_k L_k^T  (L = cov_inv_sqrt),  coef_k = pr_k * |det L_k|
  d_ik = -0.5 x^T S_k x + x^T S_k c_k - 0.5 c_k^T S_k c_k
  ll_i = log sum_k coef_k exp(d_ik) - threshold

Device strategy (data-parallel over N, 8192 points/core), all transposes
done host-side (X uploaded as X^T, duplicated across both 64-partition
halves so the PE can run two row-tiled K=64 matmuls concurrently):

  - main MM (row-tiled x2): stationary = G chunk [64, 128] (columns =
    2 clusters x 64 whitened coords), moving = X^T [64, 512 pts]
    -> Z^T psum [128, 512].  Half A (rows 0-63) and half B (rows 64-127)
    run concurrently in separate PSUM banks.
  - evac+nonlinearize, split across two engines per-tile:
      ACT route: G = L columns,  z2 = Square(Z^T)          (scalar engine)
      DVE route: G = S columns,  wz = W^T * X^T_dup        (vector engine)
    both give SBUF bf16 tiles whose 64-partition segment sums = x^T S x.
  - reduce over the 64 coords via accumulating matmuls: stationary
    S_red[m] [128, 32] holds -0.5 at (row r, col 2m + r//64); 16 chunk
    matmuls accumulate d^T [32 k, 512 pts] into a col-tiled PSUM bank
    (4 point-groups per bank at col positions 0/32/64/96).  The linear
    term x.Sc is one more accumulating matmul into the same bank.
  - epilogue: E = exp(d^T + bias_k) (ACT, per-partition bias AP),
    sum over k via ones-block matmul into a packed s bank, ll = Ln(C*s)
    with C = exp(-EXPB - threshold) folded into the activation scale.
"""

import sys

sys.path.insert(0, "/opt/trn_rl_repo")

import numpy as np

from concourse import bacc, bass, mybir
from concourse.tile import TileContext
from concourse.bass_utils import run_bass_kernel_spmd

N, D, K = 65536, 64, 32
NCORES = 8
NLOC = N // NCORES            # 8192 points per core
HALF = NLOC // 2              # 4096 points per row-tile half
PG = 512                      # points per group (psum bank width)
NPG = HALF // PG              # 8 point-groups per half
NCH = K // 2                  # 16 chunks (2 clusters each)
EXPB = 60.0                   # exp bias: exp(d + EXPB), d <= ~0 always

F32 = mybir.dt.float32
BF16 = mybir.dt.bfloat16

# wg layout (columns, bf16): [G_L2 2048 | G_S2 2048 | ScM2 32 | S_red 16*32 | ones4 4]
WG_GL = 0
WG_GS = 2048
WG_SC = 4096
WG_SRED = 4128
WG_ONES = 4128 + 16 * 32
WG_COLS = WG_ONES + 4


def _build_nc(threshold_f: float):
    nc = bacc.Bacc()

    xa_d = nc.declare_dram_parameter("xa", [128, 2 * HALF], BF16, isOutput=False)
    wg_d = nc.declare_dram_parameter("wg", [128, WG_COLS], BF16, isOutput=False)
    bias_d = nc.declare_dram_parameter("biasv", [128, 1], F32, isOutput=False)
    out_d = nc.declare_dram_parameter("out", [4, 4, PG], F32, isOutput=True)

    lnscale = float(np.exp(-EXPB) * np.exp(-threshold_f))

    with TileContext(nc) as tc:
        with (
            tc.tile_pool(name="const", bufs=1) as cpool,
            tc.tile_pool(name="z2", bufs=8) as z2pool,
            tc.tile_pool(name="ee", bufs=2) as epool,
            tc.tile_pool(name="fin", bufs=1) as finpool,
            tc.tile_pool(name="psz", bufs=5, space="PSUM") as pszpool,
            tc.tile_pool(name="psd", bufs=1, space="PSUM") as psdpool,
            tc.tile_pool(name="pss", bufs=1, space="PSUM") as psspool,
        ):
            wg = cpool.tile([128, WG_COLS], BF16)
            # G matrices first so the first main-MM can start early
            nc.sync.dma_start(out=wg[:, 0:WG_SC], in_=wg_d[:, 0:WG_SC])
            nc.sync.dma_start(out=wg[:, WG_SC:WG_COLS], in_=wg_d[:, WG_SC:WG_COLS])
            biasv = cpool.tile([128, 1], F32)
            nc.sync.dma_start(out=biasv[:, :], in_=bias_d[:, :])

            xa = cpool.tile([128, 2 * HALF], BF16)
            for s in range(8):
                nc.sync.dma_start(
                    out=xa[:, s * 1024 : (s + 1) * 1024],
                    in_=xa_d[:, s * 1024 : (s + 1) * 1024],
                )

            GL = wg[:, WG_GL : WG_GL + 2048]
            GS = wg[:, WG_GS : WG_GS + 2048]
            ScM = wg[:, WG_SC : WG_SC + 32]
            ones4 = wg[:, WG_ONES : WG_ONES + 4]

            def sred(m):
                return wg[:, WG_SRED + m * 32 : WG_SRED + (m + 1) * 32]

            sbank = psspool.tile([128, PG], F32)
            nc.vector.memset(sbank[:, :], 0.0)

            tile_idx = 0
            for gpair in range(2):
                dbankA = psdpool.tile([128, PG], F32)
                dbankB = psdpool.tile([128, PG], F32)
                nc.vector.memset(dbankA[:, :], 0.0)
                nc.vector.memset(dbankB[:, :], 0.0)
                # linear term x.Sc: 64-row x 32-col tiles, concurrent group
                for pj in range(4):
                    p = gpair * 4 + pj
                    xsA = xa[:, p * PG : (p + 1) * PG]
                    xsB = xa[:, HALF + p * PG : HALF + (p + 1) * PG]
                    nc.tensor.matmul(dbankA[32 * pj : 32 * pj + 32, :],
                                     ScM[0:64, :], xsA[0:64, :],
                                     start=False, stop=False,
                                     tile_position=(0, 32 * pj),
                                     skip_group_check=True)
                    nc.tensor.matmul(dbankB[32 * pj : 32 * pj + 32, :],
                                     ScM[64:128, :], xsB[64:128, :],
                                     start=False, stop=False,
                                     tile_position=(64, 32 * pj),
                                     skip_group_check=True)
                for m in range(16):
                    for pjp in range(2):          # sub-round: pair of pj
                        zs = []
                        for pj in (2 * pjp, 2 * pjp + 1):
                            p = gpair * 4 + pj
                            for half in (0, 1):
                                h = 64 * half
                                xs = xa[:, half * HALF + p * PG
                                        : half * HALF + (p + 1) * PG]
                                route_act = (tile_idx % 2) == 0
                                tile_idx += 1
                                G = GL if route_act else GS
                                zps = pszpool.tile([128, PG], F32)
                                nc.tensor.matmul(
                                    zps,
                                    G[h : h + 64, m * 128 : (m + 1) * 128],
                                    xs[h : h + 64, :],
                                    start=True, stop=True,
                                )
                                z2 = z2pool.tile([128, PG], BF16)
                                if route_act:
                                    nc.scalar.activation(
                                        out=z2[:, :], in_=zps[:, :],
                                        func=mybir.ActivationFunctionType.Square,
                                    )
                                else:
                                    nc.vector.tensor_tensor(
                                        out=z2[:, :], in0=zps[:, :],
                                        in1=xs[:, :],
                                        op=mybir.AluOpType.mult,
                                    )
                                zs.append((pj, half, z2))
                        # reduce: 8 (64-row x 32-col) tile MMs; group by
                        # dbank so disjoint array tiles run concurrently
                        last = (m == 15)
                        for tgt in (0, 1):
                            dbank = dbankA if tgt == 0 else dbankB
                            for pj, half, z2 in zs:
                                if half != tgt:
                                    continue
                                nc.tensor.matmul(
                                    dbank[32 * pj : 32 * pj + 32, :],
                                    sred(m), z2[:, :],
                                    start=False, stop=last,
                                    tile_position=(0, 32 * pj),
                                    skip_group_check=True,
                                )
                # epilogue for this gpair: exp, then sum over k
                for half, dbank in ((0, dbankA), (1, dbankB)):
                    E = epool.tile([128, PG], BF16)
                    nc.scalar.activation(
                        out=E[:, :], in_=dbank[:, :],
                        func=mybir.ActivationFunctionType.Exp,
                        bias=biasv[:, 0:1],
                    )
                    colpos = 32 * (2 * half + gpair)
                    nc.tensor.matmul(
                        sbank[colpos : colpos + 4, :], ones4, E[:, :],
                        start=False, stop=True,
                        tile_position=(0, colpos),
                        skip_group_check=True,
                    )

            lnb = finpool.tile([128, PG], F32)
            nc.scalar.activation(
                out=lnb[:, :], in_=sbank[:, :],
                func=mybir.ActivationFunctionType.Ln,
            )
            llf = finpool.tile([128, PG], F32)
            nc.vector.tensor_scalar(
                out=llf[:, :], in0=lnb[:, :],
                scalar1=-(EXPB + threshold_f), scalar2=None,
                op0=mybir.AluOpType.add,
            )
            for j in range(4):
                nc.sync.dma_start(
                    out=out_d[j, :, :], in_=llf[32 * j : 32 * j + 4, :],
                )

    nc.compile()
    return nc


def _host_prep(X, center, cov_inv_sqrt, weight, threshold):
    L = cov_inv_sqrt.astype(np.float64)
    w = np.abs(weight.astype(np.float64))
    pr = w / w.sum()
    sign, logdetL = np.linalg.slogdet(L)
    logcoef = np.log(pr) + logdetL                       # [K]
    S = np.einsum("kde,kfe->kdf", L, L)                  # [K, D, D]
    Sc = np.einsum("kde,ke->kd", S, center.astype(np.float64))  # [K, D]
    cSc = np.einsum("kd,kd->k", center.astype(np.float64), Sc)  # [K]

    import ml_dtypes
    BFD = ml_dtypes.bfloat16

    # G_L / G_S: [64, 2048], column k*64+c
    G_L = np.ascontiguousarray(
        L.transpose(1, 0, 2).reshape(D, K * D))         # G_L[d, k*64+c] = L[k,d,c]
    G_S = np.ascontiguousarray(
        S.transpose(1, 0, 2).reshape(D, K * D))
    wg = np.zeros((128, WG_COLS), np.float64)
    wg[0:64, WG_GL:WG_GL + 2048] = G_L
    wg[64:128, WG_GL:WG_GL + 2048] = G_L
    wg[0:64, WG_GS:WG_GS + 2048] = G_S
    wg[64:128, WG_GS:WG_GS + 2048] = G_S
    wg[0:64, WG_SC:WG_SC + 32] = Sc.T                   # ScM[c, k]
    wg[64:128, WG_SC:WG_SC + 32] = Sc.T
    for m in range(NCH):
        blk = np.zeros((128, 32))
        blk[0:64, 2 * m] = -0.5
        blk[64:128, 2 * m + 1] = -0.5
        wg[:, WG_SRED + m * 32: WG_SRED + (m + 1) * 32] = blk
    for j in range(4):
        wg[32 * j: 32 * j + 32, WG_ONES + j] = 1.0

    biasv = np.tile(EXPB + logcoef - 0.5 * cSc, 4).astype(np.float32).reshape(128, 1)

    XT = np.ascontiguousarray(X.astype(np.float64).T)   # [64, N]
    thr = float(np.asarray(threshold, dtype=np.float64))
    return XT, wg.astype(BFD), biasv, thr


_CACHE = {}


def kernel(X, center, cov_inv_sqrt, weight, threshold):
    import ml_dtypes
    BFD = ml_dtypes.bfloat16

    XT, wg, biasv, thr = _host_prep(X, center, cov_inv_sqrt, weight, threshold)

    key = ("nc", thr)
    if key not in _CACHE:
        _CACHE[key] = _build_nc(thr)
    nc = _CACHE[key]

    in_maps = []
    for i in range(NCORES):
        xt = XT[:, i * NLOC : (i + 1) * NLOC]
        xa = np.zeros((128, 2 * HALF), np.float64)
        xa[0:64, 0:HALF] = xt[:, 0:HALF]
        xa[64:128, 0:HALF] = xt[:, 0:HALF]
        xa[0:64, HALF:] = xt[:, HALF:]
        xa[64:128, HALF:] = xt[:, HALF:]
        in_maps.append({"xa": xa.astype(BFD), "wg": wg, "biasv": biasv})

    res = run_bass_kernel_spmd(nc, in_maps, core_ids=list(range(NCORES)))
    outs = res.results
    ll = np.concatenate(
        [np.asarray(outs[i]["out"], dtype=np.float32).reshape(NLOC)
         for i in range(NCORES)]
    )
    return ll
